# revision 33
# baseline (speedup 1.0000x reference)
"""Trainium2 Bass kernel for multi-head attention (B=4, N=2048, C=512, 8 heads).

Sharding: 8 cores = (batch b = core//2) x (head-group g = core%2, 4 heads each).
Per core, a transposed-scores attention pipeline:
  - host supplies x[b] transposed (xT [C, N]) and per-group transposed weights,
    all pre-cast to fp16 (matmul streams at 1 cycle/row; ~4x the mantissa of
    bf16; every tensor here fits fp16 range comfortably)
  - qT/kT stored zero-padded per head ([:, hh, :] has head hh's 64 dims on
    its own partition range, rest zero) so score matmuls contract over the
    full K=128 partition range: same N cycles as K=64, but the PE activity
    monitor sees a fully-active array and keeps the 2.4 GHz clock
  - v as [N, (1+64) per head] tiles; the leading ones column makes attn@v
    emit the softmax denominator into PSUM partition 0
  - 8 single-head sections of 16 ktok blocks. The section phase is ACT-bound
    (exp is ScalarE-only, ~1.06us per [128,1024] block, 128 blocks = ~136us
    floor) and total PE work ~= total ACT work, so the schedule's only job
    is keeping BOTH streams dense:
      * software-pipelined blocks: each iteration emits exp(i) then
        scores(i+1) then attnv(i-1) then fillers, so the next block's
        scores (the exp stream's input) is never stuck behind attnv or
        filler work, and PSUM bank waits self-correct
      * DMA loads ordered by first use (wk, xT half, wv, wq, ...), one
        trigger per token range
      * critical prefix is only kT chunk 0 + v tile 0 + qT chunks 0,1;
        everything else trickles in as deadline-scheduled fillers at
        half-chunk (256-token) granularity
      * sections run pair-grouped so pair-1 qk chunks have 3 sections of
        filler slack, and the first half of the output projection runs as
        fillers inside the last two sections
  - normalization off the PE in 512-wide chunks, recips emitted before
    broadcasts so the two chunks pipeline across DVE/GpSimd; the last
    section's chunk-0 chain gates the tail y blocks
  - tail y blocks split their PSUM eviction across ACT+DVE so the 2-deep
    PSUM ring turns over at matmul rate
  - output projection on-device; host sums the two half-head partials
"""

import sys

sys.path.insert(0, "/opt/trn_rl_repo")

import numpy as np

B, N, C = 4, 2048, 512
H, D = 8, 64
SCALE = float(D) ** -0.5  # 0.125, exact in fp32
P = 128
CT = C // P  # 4 contraction tiles over channels
NT = N // P  # 16 token blocks
NCORES = 8
FD = 1024  # softmax block free dim (q chunk)
QH = N // FD  # 2 q halves

_cache = {}


def _build():
    import concourse.bacc as bacc
    import concourse.tile as tile
    from concourse import mybir

    f32 = mybir.dt.float32
    f16 = mybir.dt.float16
    u16 = mybir.dt.uint16
    EXP = mybir.ActivationFunctionType.Exp

    nc = bacc.Bacc("TRN2", target_bir_lowering=False, debug=False,
                   num_devices=NCORES)

    xT_d = nc.dram_tensor("xT", [C, N], f16, kind="ExternalInput")
    wqT_d = nc.dram_tensor("wqT", [P, CT * 256], f16, kind="ExternalInput")
    wkT_d = nc.dram_tensor("wkT", [P, CT * 256], f16, kind="ExternalInput")
    wvT_d = nc.dram_tensor("wvT", [P, CT * 256], f16, kind="ExternalInput")
    pwT_d = nc.dram_tensor("pwT", [P, 2 * C], f16, kind="ExternalInput")
    y_d = nc.dram_tensor("y", [N, C], f32, kind="ExternalOutput")

    with tile.TileContext(nc) as tc:
        with (
            tc.tile_pool(name="io", bufs=1) as io,
            tc.tile_pool(name="qk", bufs=1) as qk,
            tc.tile_pool(name="expp", bufs=6) as expp,
            tc.tile_pool(name="workp", bufs=3) as workp,
            tc.tile_pool(name="yp", bufs=4) as yp,
            tc.tile_pool(name="ps_s", bufs=2, space="PSUM") as ps_s,
            tc.tile_pool(name="ps_o", bufs=1, space="PSUM") as ps_o,
        ):
            # ---- input loads, ordered by first use ----
            xT_sb = io.tile([P, CT, N], f16, tag="xT", name="xT_sb")
            xT_ap = xT_d[:].rearrange("(t p) n -> p t n", p=P)
            wk_sb = io.tile([P, CT, 256], f16, tag="wk", name="wk_sb")
            nc.sync.dma_start(
                wk_sb[:], wkT_d[:].rearrange("p (t m) -> p t m", t=CT))
            nc.sync.dma_start(xT_sb[:, :, 0:256], xT_ap[:, :, 0:256])
            wq_sb = io.tile([P, CT, 256], f16, tag="wq", name="wq_sb")
            nc.sync.dma_start(
                wq_sb[:], wqT_d[:].rearrange("p (t m) -> p t m", t=CT))
            nc.sync.dma_start(xT_sb[:, :, 256:512], xT_ap[:, :, 256:512])
            # bulk xT in t-split pairs so two DMA queues stream concurrently
            nc.sync.dma_start(xT_sb[:, 0:2, 512:1536], xT_ap[:, 0:2, 512:1536])
            nc.sync.dma_start(xT_sb[:, 2:4, 512:1536], xT_ap[:, 2:4, 512:1536])
            wv_sb = io.tile([P, CT, 256], f16, tag="wv", name="wv_sb")
            nc.sync.dma_start(
                wv_sb[:], wvT_d[:].rearrange("p (t m) -> p t m", t=CT))
            nc.sync.dma_start(xT_sb[:, 0:2, 1536:2048], xT_ap[:, 0:2, 1536:2048])
            nc.sync.dma_start(xT_sb[:, 2:4, 1536:2048], xT_ap[:, 2:4, 1536:2048])
            pw_sb = io.tile([P, 2, C], f16, tag="pw", name="pw_sb")
            nc.sync.dma_start(
                pw_sb[:], pwT_d[:].rearrange("p (t m) -> p t m", t=2))

            # ---- SBUF persistents ----
            qT = []
            kT = []
            vv = []
            outT = []
            for p in range(2):
                qT.append(qk.tile([P, 2, N], f16, tag=f"qT{p}", name=f"qT{p}"))
                kT.append(qk.tile([P, 2, N], f16, tag=f"kT{p}", name=f"kT{p}"))
                vv.append(qk.tile([P, NT, 130], f16, tag=f"v{p}", name=f"v{p}"))
                outT.append(qk.tile([P, N], f16, tag=f"outT{p}", name=f"outT{p}"))

            # trigger the ACT exp table load during the DMA ramp
            scratch1 = io.tile([1, 2], f32, tag="scratch1", name="scratch1")
            nc.vector.memset(scratch1[:], 0.0)
            nc.scalar.activation(scratch1[0:1, 0:1], scratch1[0:1, 1:2], EXP)
            # all pads on GpSimd (its queue has nothing else early, so they
            # run right at kernel start; DVE stays free for the prefix
            # evictions), slot-0 halves first
            nc.gpsimd.memset(kT[0][64:128, 0, :], 0.0)
            nc.gpsimd.memset(qT[0][64:128, 0, :], 0.0)
            nc.gpsimd.memset(kT[0][0:64, 1, :], 0.0)
            nc.gpsimd.memset(qT[0][0:64, 1, :], 0.0)
            for p in range(2):
                # ones columns (fp16 1.0) at the head of each v block (DVE:
                # strided 16-element memsets are fast there, slow on GpSimd)
                nc.vector.memset(vv[p][:, :, 0:1].bitcast(u16), 0x3C00)
                nc.vector.memset(vv[p][:, :, 65:66].bitcast(u16), 0x3C00)

            # PSUM layout: tag "s" = the scores<->exp ring (2x2 banks), tag
            # "f" = everything else (prefix chunks, v tiles, y blocks; 2x1
            # bank), tag "o" = the attn@v accumulator (1x2 banks). Separate
            # rings keep filler eviction latency out of the exp pipeline.
            def emit_qk_chunk(p, w_sb, dst, cs, act_evict=False, ring="f"):
                # cs: token slice (512-wide prefix chunks, 256-wide fillers)
                cw = cs.stop - cs.start
                pc = slice(128 * p, 128 * (p + 1))
                shape = [P, 512] if ring == "f" else [P, FD]
                ps = ps_s.tile(shape, f32, tag=ring,
                               name=f"qkps_{p}_{cs.start}_{w_sb.tensor.name}")
                for t in range(CT):
                    nc.tensor.matmul(
                        ps[:, :cw],
                        lhsT=w_sb[:, t, pc],
                        rhs=xT_sb[:, t, cs],
                        start=(t == 0), stop=(t == CT - 1))
                nc.vector.tensor_copy(dst[0:64, 0, cs], ps[0:64, :cw])
                if act_evict:
                    nc.scalar.copy(dst[64:128, 1, cs], ps[64:128, :cw])
                else:
                    nc.vector.tensor_copy(dst[64:128, 1, cs], ps[64:128, :cw])

            def emit_v_tile(tt):
                psv = ps_s.tile([P, 512], f32, tag="f", name=f"vps_{tt}")
                for t in range(CT):
                    nc.tensor.matmul(
                        psv[:, :256],
                        lhsT=xT_sb[:, t, 128 * tt:128 * (tt + 1)],
                        rhs=wv_sb[:, t, 0:256],
                        start=(t == 0), stop=(t == CT - 1))
                for p in range(2):
                    pv = psv[:, 128 * p:128 * (p + 1)].rearrange(
                        "p (two d) -> p two d", two=2)
                    dv = vv[p][:, tt, 0:130].rearrange(
                        "p (two d65) -> p two d65", two=2)[:, :, 1:65]
                    nc.vector.tensor_copy(dv, pv)

            def emit_y_block(tt, mode):
                yps = ps_s.tile([P, 512], f32, tag="f", name=f"yps_{tt}")
                for p in range(2):
                    nc.tensor.matmul(
                        yps[:, :512], lhsT=outT[p][:, 128 * tt:128 * (tt + 1)],
                        rhs=pw_sb[:, p, :], start=(p == 0), stop=(p == 1))
                ys = yp.tile([P, C], f32, tag="y", name=f"ys_{tt}")
                if mode == "dual":
                    # tail: split across ACT+DVE so the PSUM ring turns over
                    # at matmul rate
                    nc.scalar.copy(ys[:, 0:256], yps[:, 0:256])
                    nc.vector.tensor_copy(ys[:, 256:512], yps[:, 256:512])
                elif mode == "act":
                    nc.scalar.copy(ys[:], yps[:, :512])
                else:
                    nc.vector.tensor_copy(ys[:], yps[:, :512])
                nc.sync.dma_start(y_d[128 * tt:128 * (tt + 1), :], ys[:])

            def norm_head(p, hh, qh, o, last=False):
                # evict o to SBUF immediately (frees the single-buffered o
                # bank for the next section's attn@v), then normalize from
                # the copy: DVE recips -> GpSimd broadcasts -> DVE muls.
                # The last section skips the staging copy (its o bank has no
                # next user) to shorten the gate on the tail y blocks.
                if last:
                    oc = o
                else:
                    oc = workp.tile([65, FD], f32, tag="oc",
                                    name=f"oc_{p}_{hh}_{qh}")
                    nc.vector.tensor_copy(oc[:], o[:])
                rs = []
                for c in range(2):
                    r = workp.tile([1, 512], f32, tag="r",
                                   name=f"r_{p}_{hh}_{qh}_{c}")
                    nc.vector.reciprocal_approx_fast(
                        r[0:1, :], oc[0:1, 512 * c:512 * (c + 1)])
                    rs.append(r)
                rbs = []
                for c in range(2):
                    rb = workp.tile([65, 512], f32, tag="rb",
                                    name=f"rb_{p}_{hh}_{qh}_{c}")
                    nc.gpsimd.partition_broadcast(rb[:], rs[c][0:1, :])
                    rbs.append(rb)
                for c in range(2):
                    qs = slice(FD * qh + 512 * c, FD * qh + 512 * (c + 1))
                    st = workp.tile([65, 512], f16, tag="st",
                                    name=f"st_{p}_{hh}_{qh}_{c}")
                    nc.vector.tensor_mul(
                        st[:], oc[:, 512 * c:512 * (c + 1)], rbs[c][:])
                    nc.sync.dma_start(outT[p][64 * hh:64 * (hh + 1), qs],
                                      st[1:65, :])

            # deadline-scheduled fillers: (section_idx, block_idx) -> [fn]
            fill = {}

            def add_fill(si, blk, fn):
                fill.setdefault((si, blk), []).append(fn)

            def emit_section(si, p, hh, qh, last=False):
                vs = slice(65 * hh, 65 * (hh + 1))
                o = ps_o.tile([65, FD], f32, tag="o", name=f"o_{p}_{hh}_{qh}")

                def emit_scores(i):
                    ks = slice(128 * i, 128 * (i + 1))
                    s = ps_s.tile([P, FD], f32, tag="s",
                                  name=f"s_{p}_{hh}_{qh}_{i}")
                    for j in range(2):
                        js = slice(512 * j, 512 * (j + 1))
                        qj = slice(FD * qh + 512 * j, FD * qh + 512 * (j + 1))
                        nc.tensor.matmul(
                            s[:, js], lhsT=kT[p][:, hh, ks],
                            rhs=qT[p][:, hh, qj], start=True, stop=True)
                    return s

                def emit_attnv(i, e):
                    for j in range(2):
                        js = slice(512 * j, 512 * (j + 1))
                        nc.tensor.matmul(
                            o[:, js], lhsT=vv[p][:, i, vs], rhs=e[:, js],
                            start=(i == 0), stop=(i == NT - 1))

                s_cur = emit_scores(0)
                e_prev = None
                for i in range(NT):
                    e_cur = expp.tile([P, FD], f16, tag="exp",
                                      name=f"e_{p}_{hh}_{qh}_{i}")
                    nc.scalar.activation(e_cur[:], s_cur[:], EXP)
                    if i + 1 < NT:
                        s_cur = emit_scores(i + 1)
                    if e_prev is not None:
                        emit_attnv(i - 1, e_prev)
                    e_prev = e_cur
                    for fn in fill.pop((si, i), []):
                        fn()
                emit_attnv(NT - 1, e_prev)

                norm_head(p, hh, qh, o, last=last)

            # critical prefix: only what section (0,0,0) block 0 touches,
            # quarter-chunk granularity ordered by DMA arrival. scores
            # block 0 only needs kT tokens 0:128; the rest of kT[0] fills
            # in deadline order inside section 0.
            # prefix chunks alternate PSUM rings (f/s — scores haven't
            # started, so the s ring is free) so each chunk's matmuls never
            # wait on the immediately preceding chunk's evictions
            emit_qk_chunk(0, wk_sb, kT[0], slice(0, 128))
            emit_qk_chunk(0, wq_sb, qT[0], slice(0, 256), ring="s")
            emit_qk_chunk(0, wk_sb, kT[0], slice(128, 384))
            emit_qk_chunk(0, wq_sb, qT[0], slice(256, 512), ring="s")
            emit_qk_chunk(0, wq_sb, qT[0], slice(512, 768))
            emit_qk_chunk(0, wq_sb, qT[0], slice(768, 1024), ring="s")
            emit_v_tile(0)

            def hc(p, w_sb, dst, h):  # 256-token half-chunk filler
                return lambda: emit_qk_chunk(
                    p, w_sb, dst, slice(256 * h, 256 * (h + 1)))

            # section 0: v tiles 1-15 (tile tt consumed by attnv at block
            # tt+1); kT[0] pieces (tokens 256h+128, needed by scores block
            # 2h+1 which is emitted at iteration 2h) placed at iteration
            # 2h-1
            for tt in range(1, NT):
                add_fill(0, tt, lambda tt=tt: emit_v_tile(tt))
            for h in range(1, 7):
                add_fill(0, 2 * h - 1, lambda h=h: emit_qk_chunk(
                    0, wk_sb, kT[0], slice(256 * h + 128, 256 * h + 384)))
            add_fill(0, 13, lambda: emit_qk_chunk(
                0, wk_sb, kT[0], slice(1920, 2048)))
            # section 1: qT[0] tokens 1024:2048 (needed by section 2 =
            # (0,0,1)), pair-1 pads (gpsimd), first pair-1 k half-chunks
            add_fill(1, 1, lambda: nc.gpsimd.memset(kT[1][64:128, 0, :], 0.0))
            add_fill(1, 2, lambda: nc.gpsimd.memset(kT[1][0:64, 1, :], 0.0))
            add_fill(1, 3, lambda: nc.gpsimd.memset(qT[1][64:128, 0, :], 0.0))
            add_fill(1, 4, lambda: nc.gpsimd.memset(qT[1][0:64, 1, :], 0.0))
            for j, h in enumerate(range(4, 8)):
                add_fill(1, 2 + 3 * j, hc(0, wq_sb, qT[0], h))
            add_fill(1, 13, hc(1, wk_sb, kT[1], 0))
            add_fill(1, 15, hc(1, wk_sb, kT[1], 1))
            # sections 2-3: rest of pair-1 (deadline: section 4 start)
            for j, h in enumerate(range(2, 8)):
                add_fill(2, 1 + 2 * j, hc(1, wk_sb, kT[1], h))
            add_fill(2, 13, hc(1, wq_sb, qT[1], 0))
            add_fill(2, 15, hc(1, wq_sb, qT[1], 1))
            for j, h in enumerate(range(2, 8)):
                add_fill(3, 1 + 2 * j, hc(1, wq_sb, qT[1], h))
            # sections 6-7: y blocks 0-7 (qh=0 outT complete after section
            # 5's norm chain ~5us into section 6 — placed mid-section so the
            # in-order PE never parks on a not-yet-normalized outT; DVE
            # evictions keep ACT pure-exp)
            for i in range(8):
                add_fill(6, 5 + ((10 * i) // 7),
                         lambda tt=i: emit_y_block(tt, "dve"))

            # pair-grouped section order: pair-1 qk chunks get 3 sections of
            # filler slack; last two sections host the first 8 y blocks
            sections = [(0, 0, 0), (0, 1, 0), (0, 0, 1), (0, 1, 1),
                        (1, 0, 0), (1, 1, 0), (1, 0, 1), (1, 1, 1)]
            for si, (p, hh, qh) in enumerate(sections):
                emit_section(si, p, hh, qh, last=(si == len(sections) - 1))
            for key in sorted(fill.keys()):
                for fn in fill.pop(key):
                    fn()

            # ---- tail: y blocks 8-15 ----
            for tt in range(8, NT):
                emit_y_block(tt, "dual")

    nc.finalize()
    return nc


def _get_nc():
    if "nc" not in _cache:
        _cache["nc"] = _build()
    return _cache["nc"]


def _pack(wt, groups):
    # [G*128, M] row-major -> [128, G*M]: partition p holds the concat over
    # groups of row (g*128 + p), so the DMA reads one contiguous run per p
    g128, m = wt.shape
    assert g128 == groups * 128
    return np.ascontiguousarray(
        wt.reshape(groups, 128, m).transpose(1, 0, 2).reshape(128, groups * m))


def _make_in_maps(x, q_w, kv_w, proj_w):
    x = np.asarray(x, dtype=np.float32)
    q_w = np.asarray(q_w, dtype=np.float32)
    kv_w = np.asarray(kv_w, dtype=np.float32)
    proj_w = np.asarray(proj_w, dtype=np.float32)
    f16 = np.float16
    in_maps = []
    for core in range(NCORES):
        b, g = core // 2, core % 2
        hs = slice(g * 256, (g + 1) * 256)
        in_maps.append({
            "xT": np.ascontiguousarray(x[b].T.astype(f16)),
            "wqT": _pack((q_w[hs, :] * np.float32(SCALE)).T.astype(f16), CT),
            "wkT": _pack(kv_w[hs, :].T.astype(f16), CT),
            "wvT": _pack(
                kv_w[C + g * 256:C + (g + 1) * 256, :].T.astype(f16), CT),
            "pwT": _pack(proj_w[:, hs].T.astype(f16), 2),
        })
    return in_maps


def kernel(x, q_w, kv_w, proj_w, proj_b, H=None, W=None, _trace=False):
    from concourse.bass_utils import run_bass_kernel_spmd

    nc = _get_nc()
    in_maps = _make_in_maps(x, q_w, kv_w, proj_w)
    res = run_bass_kernel_spmd(nc, in_maps, core_ids=list(range(NCORES)),
                               trace=_trace)
    proj_b = np.asarray(proj_b, dtype=np.float32)
    out = np.empty((B, N, C), dtype=np.float32)
    for b in range(B):
        out[b] = res.results[2 * b]["y"] + res.results[2 * b + 1]["y"] + proj_b
    if _trace:
        return out, res
    return out


# revision 34
# speedup vs baseline: 1.0240x; 1.0240x over previous
"""Trainium2 Bass kernel for multi-head attention (B=4, N=2048, C=512, 8 heads).

Sharding: 8 cores = (batch b = core//2) x (head-group g = core%2, 4 heads each).
Per core, a transposed-scores attention pipeline:
  - host supplies x[b] transposed (xT [C, N]) and per-group transposed weights,
    all pre-cast to fp16 (matmul streams at 1 cycle/row; ~4x the mantissa of
    bf16; every tensor here fits fp16 range comfortably)
  - qT/kT stored zero-padded per head ([:, hh, :] has head hh's 64 dims on
    its own partition range, rest zero) so score matmuls contract over the
    full K=128 partition range: same N cycles as K=64, but the PE activity
    monitor sees a fully-active array and keeps the 2.4 GHz clock
  - v as [N, (1+64) per head] tiles; the leading ones column makes attn@v
    emit the softmax denominator into PSUM partition 0
  - 8 single-head sections of 16 ktok blocks. The section phase is ACT-bound
    (exp is ScalarE-only, ~1.06us per [128,1024] block, 128 blocks = ~136us
    floor) and total PE work ~= total ACT work, so the schedule's only job
    is keeping BOTH streams dense:
      * software-pipelined blocks: each iteration emits exp(i) then
        scores(i+1) then attnv(i-1) then fillers, so the next block's
        scores (the exp stream's input) is never stuck behind attnv or
        filler work, and PSUM bank waits self-correct
      * DMA loads ordered by first use (wk, xT half, wv, wq, ...), one
        trigger per token range
      * critical prefix is only kT chunk 0 + v tile 0 + qT chunks 0,1;
        everything else trickles in as deadline-scheduled fillers at
        half-chunk (256-token) granularity
      * sections run pair-grouped so pair-1 qk chunks have 3 sections of
        filler slack, and the first half of the output projection runs as
        fillers inside the last two sections
  - normalization off the PE in 512-wide chunks, recips emitted before
    broadcasts so the two chunks pipeline across DVE/GpSimd; the last
    section's chunk-0 chain gates the tail y blocks
  - tail y blocks split their PSUM eviction across ACT+DVE so the 2-deep
    PSUM ring turns over at matmul rate
  - output projection on-device; host sums the two half-head partials
"""

import sys

sys.path.insert(0, "/opt/trn_rl_repo")

import numpy as np

B, N, C = 4, 2048, 512
H, D = 8, 64
SCALE = float(D) ** -0.5  # 0.125, exact in fp32
P = 128
CT = C // P  # 4 contraction tiles over channels
NT = N // P  # 16 token blocks
NCORES = 8
FD = 1024  # softmax block free dim (q chunk)
QH = N // FD  # 2 q halves

_cache = {}


def _build():
    import concourse.bacc as bacc
    import concourse.tile as tile
    from concourse import mybir

    f32 = mybir.dt.float32
    f16 = mybir.dt.float16
    u16 = mybir.dt.uint16
    EXP = mybir.ActivationFunctionType.Exp

    nc = bacc.Bacc("TRN2", target_bir_lowering=False, debug=False,
                   num_devices=NCORES)

    xT_d = nc.dram_tensor("xT", [C, N], f16, kind="ExternalInput")
    wqT_d = nc.dram_tensor("wqT", [P, CT * 256], f16, kind="ExternalInput")
    wkT_d = nc.dram_tensor("wkT", [P, CT * 256], f16, kind="ExternalInput")
    wvT_d = nc.dram_tensor("wvT", [P, CT * 256], f16, kind="ExternalInput")
    pwT_d = nc.dram_tensor("pwT", [P, 2 * C], f16, kind="ExternalInput")
    y_d = nc.dram_tensor("y", [N, C], f32, kind="ExternalOutput")

    with tile.TileContext(nc) as tc:
        with (
            tc.tile_pool(name="io", bufs=1) as io,
            tc.tile_pool(name="qk", bufs=1) as qk,
            tc.tile_pool(name="expp", bufs=6) as expp,
            tc.tile_pool(name="workp", bufs=3) as workp,
            tc.tile_pool(name="yp", bufs=4) as yp,
            tc.tile_pool(name="ps_s", bufs=2, space="PSUM") as ps_s,
            tc.tile_pool(name="ps_o", bufs=1, space="PSUM") as ps_o,
        ):
            # ---- input loads, ordered by first use ----
            xT_sb = io.tile([P, CT, N], f16, tag="xT", name="xT_sb")
            xT_ap = xT_d[:].rearrange("(t p) n -> p t n", p=P)
            wk_sb = io.tile([P, CT, 256], f16, tag="wk", name="wk_sb")
            nc.sync.dma_start(
                wk_sb[:], wkT_d[:].rearrange("p (t m) -> p t m", t=CT))
            nc.sync.dma_start(xT_sb[:, :, 0:256], xT_ap[:, :, 0:256])
            wq_sb = io.tile([P, CT, 256], f16, tag="wq", name="wq_sb")
            nc.sync.dma_start(
                wq_sb[:], wqT_d[:].rearrange("p (t m) -> p t m", t=CT))
            nc.sync.dma_start(xT_sb[:, :, 256:512], xT_ap[:, :, 256:512])
            # bulk xT in t-split pairs so two DMA queues stream concurrently
            nc.sync.dma_start(xT_sb[:, 0:2, 512:1536], xT_ap[:, 0:2, 512:1536])
            nc.sync.dma_start(xT_sb[:, 2:4, 512:1536], xT_ap[:, 2:4, 512:1536])
            wv_sb = io.tile([P, CT, 256], f16, tag="wv", name="wv_sb")
            nc.sync.dma_start(
                wv_sb[:], wvT_d[:].rearrange("p (t m) -> p t m", t=CT))
            nc.sync.dma_start(xT_sb[:, 0:2, 1536:2048], xT_ap[:, 0:2, 1536:2048])
            nc.sync.dma_start(xT_sb[:, 2:4, 1536:2048], xT_ap[:, 2:4, 1536:2048])
            pw_sb = io.tile([P, 2, C], f16, tag="pw", name="pw_sb")
            nc.sync.dma_start(
                pw_sb[:], pwT_d[:].rearrange("p (t m) -> p t m", t=2))

            # ---- SBUF persistents ----
            qT = []
            kT = []
            vv = []
            outT = []
            for p in range(2):
                qT.append(qk.tile([P, 2, N], f16, tag=f"qT{p}", name=f"qT{p}"))
                kT.append(qk.tile([P, 2, N], f16, tag=f"kT{p}", name=f"kT{p}"))
                vv.append(qk.tile([P, NT, 130], f16, tag=f"v{p}", name=f"v{p}"))
                outT.append(qk.tile([P, N], f16, tag=f"outT{p}", name=f"outT{p}"))

            # trigger the ACT exp table load during the DMA ramp
            scratch1 = io.tile([1, 2], f32, tag="scratch1", name="scratch1")
            nc.vector.memset(scratch1[:], 0.0)
            nc.scalar.activation(scratch1[0:1, 0:1], scratch1[0:1, 1:2], EXP)
            # all pads on GpSimd (its queue has nothing else early, so they
            # run right at kernel start; DVE stays free for the prefix
            # evictions), slot-0 halves first
            nc.gpsimd.memset(kT[0][64:128, 0, :], 0.0)
            nc.gpsimd.memset(qT[0][64:128, 0, :], 0.0)
            nc.gpsimd.memset(kT[0][0:64, 1, :], 0.0)
            nc.gpsimd.memset(qT[0][0:64, 1, :], 0.0)
            for p in range(2):
                # ones columns (fp16 1.0) at the head of each v block (DVE:
                # strided 16-element memsets are fast there, slow on GpSimd)
                nc.vector.memset(vv[p][:, :, 0:1].bitcast(u16), 0x3C00)
                nc.vector.memset(vv[p][:, :, 65:66].bitcast(u16), 0x3C00)

            # PSUM layout: tag "s" = the scores<->exp ring (2x2 banks), tag
            # "f" = everything else (prefix chunks, v tiles, y blocks; 2x1
            # bank), tag "o" = the attn@v accumulator (1x2 banks). Separate
            # rings keep filler eviction latency out of the exp pipeline.
            def emit_qk_chunk(p, w_sb, dst, cs, act_evict=False, ring="f"):
                # cs: token slice (512-wide prefix chunks, 256-wide fillers)
                cw = cs.stop - cs.start
                pc = slice(128 * p, 128 * (p + 1))
                shape = [P, 512] if ring == "f" else [P, FD]
                ps = ps_s.tile(shape, f32, tag=ring,
                               name=f"qkps_{p}_{cs.start}_{w_sb.tensor.name}")
                for t in range(CT):
                    nc.tensor.matmul(
                        ps[:, :cw],
                        lhsT=w_sb[:, t, pc],
                        rhs=xT_sb[:, t, cs],
                        start=(t == 0), stop=(t == CT - 1))
                nc.vector.tensor_copy(dst[0:64, 0, cs], ps[0:64, :cw])
                if act_evict:
                    nc.scalar.copy(dst[64:128, 1, cs], ps[64:128, :cw])
                else:
                    nc.vector.tensor_copy(dst[64:128, 1, cs], ps[64:128, :cw])

            def emit_v_tile(tt):
                psv = ps_s.tile([P, 512], f32, tag="f", name=f"vps_{tt}")
                for t in range(CT):
                    nc.tensor.matmul(
                        psv[:, :256],
                        lhsT=xT_sb[:, t, 128 * tt:128 * (tt + 1)],
                        rhs=wv_sb[:, t, 0:256],
                        start=(t == 0), stop=(t == CT - 1))
                for p in range(2):
                    pv = psv[:, 128 * p:128 * (p + 1)].rearrange(
                        "p (two d) -> p two d", two=2)
                    dv = vv[p][:, tt, 0:130].rearrange(
                        "p (two d65) -> p two d65", two=2)[:, :, 1:65]
                    nc.vector.tensor_copy(dv, pv)

            def emit_y_block(tt, mode):
                yps = ps_s.tile([P, 512], f32, tag="f", name=f"yps_{tt}")
                for p in range(2):
                    nc.tensor.matmul(
                        yps[:, :512], lhsT=outT[p][:, 128 * tt:128 * (tt + 1)],
                        rhs=pw_sb[:, p, :], start=(p == 0), stop=(p == 1))
                ys = yp.tile([P, C], f32, tag="y", name=f"ys_{tt}")
                if mode == "dual":
                    # tail: split across ACT+DVE so the PSUM ring turns over
                    # at matmul rate
                    nc.scalar.copy(ys[:, 0:256], yps[:, 0:256])
                    nc.vector.tensor_copy(ys[:, 256:512], yps[:, 256:512])
                elif mode == "act":
                    nc.scalar.copy(ys[:], yps[:, :512])
                else:
                    nc.vector.tensor_copy(ys[:], yps[:, :512])
                nc.sync.dma_start(y_d[128 * tt:128 * (tt + 1), :], ys[:])

            def norm_head(p, hh, qh, o, last=False):
                # evict o to SBUF immediately (frees the single-buffered o
                # bank for the next section's attn@v), then normalize from
                # the copy: DVE recips -> GpSimd broadcasts -> DVE muls.
                # The last section skips the staging copy (its o bank has no
                # next user) and runs 256-wide chunks so the first tail y
                # blocks' outT slice lands as early as possible.
                if last:
                    oc = o
                    nch, cw = 4, 256
                else:
                    oc = workp.tile([65, FD], f32, tag="oc",
                                    name=f"oc_{p}_{hh}_{qh}")
                    nc.vector.tensor_copy(oc[:], o[:])
                    nch, cw = 2, 512
                rs = []
                for c in range(nch):
                    r = workp.tile([1, 512], f32, tag="r",
                                   name=f"r_{p}_{hh}_{qh}_{c}", bufs=4)
                    nc.vector.reciprocal_approx_fast(
                        r[0:1, :cw], oc[0:1, cw * c:cw * (c + 1)])
                    rs.append(r)
                rbs = []
                for c in range(nch):
                    rb = workp.tile([65, 512], f32, tag="rb",
                                    name=f"rb_{p}_{hh}_{qh}_{c}", bufs=4)
                    nc.gpsimd.partition_broadcast(rb[:, :cw], rs[c][0:1, :cw])
                    rbs.append(rb)
                for c in range(nch):
                    qs = slice(FD * qh + cw * c, FD * qh + cw * (c + 1))
                    st = workp.tile([65, 512], f16, tag="st",
                                    name=f"st_{p}_{hh}_{qh}_{c}", bufs=4)
                    nc.vector.tensor_mul(
                        st[:, :cw], oc[:, cw * c:cw * (c + 1)], rbs[c][:, :cw])
                    nc.sync.dma_start(outT[p][64 * hh:64 * (hh + 1), qs],
                                      st[1:65, :cw])

            # deadline-scheduled fillers: (section_idx, block_idx) -> [fn]
            fill = {}

            def add_fill(si, blk, fn):
                fill.setdefault((si, blk), []).append(fn)

            def emit_section(si, p, hh, qh, last=False):
                vs = slice(65 * hh, 65 * (hh + 1))
                o = ps_o.tile([65, FD], f32, tag="o", name=f"o_{p}_{hh}_{qh}")

                def emit_scores(i):
                    ks = slice(128 * i, 128 * (i + 1))
                    s = ps_s.tile([P, FD], f32, tag="s",
                                  name=f"s_{p}_{hh}_{qh}_{i}")
                    for j in range(2):
                        js = slice(512 * j, 512 * (j + 1))
                        qj = slice(FD * qh + 512 * j, FD * qh + 512 * (j + 1))
                        nc.tensor.matmul(
                            s[:, js], lhsT=kT[p][:, hh, ks],
                            rhs=qT[p][:, hh, qj], start=True, stop=True)
                    return s

                def emit_attnv(i, e):
                    for j in range(2):
                        js = slice(512 * j, 512 * (j + 1))
                        nc.tensor.matmul(
                            o[:, js], lhsT=vv[p][:, i, vs], rhs=e[:, js],
                            start=(i == 0), stop=(i == NT - 1))

                s_cur = emit_scores(0)
                e_prev = None
                for i in range(NT):
                    e_cur = expp.tile([P, FD], f16, tag="exp",
                                      name=f"e_{p}_{hh}_{qh}_{i}")
                    nc.scalar.activation(e_cur[:], s_cur[:], EXP)
                    if i + 1 < NT:
                        s_cur = emit_scores(i + 1)
                    if e_prev is not None:
                        emit_attnv(i - 1, e_prev)
                    e_prev = e_cur
                    for fn in fill.pop((si, i), []):
                        fn()
                emit_attnv(NT - 1, e_prev)

                norm_head(p, hh, qh, o, last=last)

            # critical prefix: only what section (0,0,0) block 0 touches,
            # quarter-chunk granularity ordered by DMA arrival. scores
            # block 0 only needs kT tokens 0:128; the rest of kT[0] fills
            # in deadline order inside section 0.
            # prefix chunks alternate PSUM rings (f/s — scores haven't
            # started, so the s ring is free) so each chunk's matmuls never
            # wait on the immediately preceding chunk's evictions
            emit_qk_chunk(0, wk_sb, kT[0], slice(0, 128))
            emit_qk_chunk(0, wq_sb, qT[0], slice(0, 256), ring="s")
            emit_qk_chunk(0, wk_sb, kT[0], slice(128, 384))
            emit_qk_chunk(0, wq_sb, qT[0], slice(256, 512), ring="s")
            emit_qk_chunk(0, wq_sb, qT[0], slice(512, 768))
            emit_qk_chunk(0, wq_sb, qT[0], slice(768, 1024), ring="s")
            emit_v_tile(0)

            def hc(p, w_sb, dst, h):  # 256-token half-chunk filler
                return lambda: emit_qk_chunk(
                    p, w_sb, dst, slice(256 * h, 256 * (h + 1)))

            # section 0: v tiles 1-15 (tile tt consumed by attnv at block
            # tt+1); kT[0] pieces (tokens 256h+128, needed by scores block
            # 2h+1 which is emitted at iteration 2h) placed at iteration
            # 2h-1
            for tt in range(1, NT):
                add_fill(0, tt, lambda tt=tt: emit_v_tile(tt))
            for h in range(1, 7):
                add_fill(0, 2 * h - 1, lambda h=h: emit_qk_chunk(
                    0, wk_sb, kT[0], slice(256 * h + 128, 256 * h + 384)))
            add_fill(0, 13, lambda: emit_qk_chunk(
                0, wk_sb, kT[0], slice(1920, 2048)))
            # section 1: qT[0] tokens 1024:2048 (needed by section 2 =
            # (0,0,1)), pair-1 pads (gpsimd), first pair-1 k half-chunks
            add_fill(1, 1, lambda: nc.gpsimd.memset(kT[1][64:128, 0, :], 0.0))
            add_fill(1, 2, lambda: nc.gpsimd.memset(kT[1][0:64, 1, :], 0.0))
            add_fill(1, 3, lambda: nc.gpsimd.memset(qT[1][64:128, 0, :], 0.0))
            add_fill(1, 4, lambda: nc.gpsimd.memset(qT[1][0:64, 1, :], 0.0))
            for j, h in enumerate(range(4, 8)):
                add_fill(1, 2 + 3 * j, hc(0, wq_sb, qT[0], h))
            add_fill(1, 13, hc(1, wk_sb, kT[1], 0))
            add_fill(1, 15, hc(1, wk_sb, kT[1], 1))
            # sections 2-3: rest of pair-1 (deadline: section 4 start)
            for j, h in enumerate(range(2, 8)):
                add_fill(2, 1 + 2 * j, hc(1, wk_sb, kT[1], h))
            add_fill(2, 13, hc(1, wq_sb, qT[1], 0))
            add_fill(2, 15, hc(1, wq_sb, qT[1], 1))
            for j, h in enumerate(range(2, 8)):
                add_fill(3, 1 + 2 * j, hc(1, wq_sb, qT[1], h))
            # sections 6-7: y blocks 0-7 (qh=0 outT complete after section
            # 5's norm chain ~5us into section 6 — placed mid-section so the
            # in-order PE never parks on a not-yet-normalized outT; DVE
            # evictions keep ACT pure-exp)
            for i in range(4):
                add_fill(6, 6 + 3 * i, lambda tt=i: emit_y_block(tt, "dve"))
            for i in range(4, 8):
                add_fill(7, 2 + 3 * (i - 4),
                         lambda tt=i: emit_y_block(tt, "dve"))

            # pair-grouped section order: pair-1 qk chunks get 3 sections of
            # filler slack; last two sections host the first 8 y blocks
            sections = [(0, 0, 0), (0, 1, 0), (0, 0, 1), (0, 1, 1),
                        (1, 0, 0), (1, 1, 0), (1, 0, 1), (1, 1, 1)]
            for si, (p, hh, qh) in enumerate(sections):
                emit_section(si, p, hh, qh, last=(si == len(sections) - 1))
            for key in sorted(fill.keys()):
                for fn in fill.pop(key):
                    fn()

            # ---- tail: y blocks 8-15 ----
            for tt in range(8, NT):
                emit_y_block(tt, "dual")

    nc.finalize()
    return nc


def _get_nc():
    if "nc" not in _cache:
        _cache["nc"] = _build()
    return _cache["nc"]


def _pack(wt, groups):
    # [G*128, M] row-major -> [128, G*M]: partition p holds the concat over
    # groups of row (g*128 + p), so the DMA reads one contiguous run per p
    g128, m = wt.shape
    assert g128 == groups * 128
    return np.ascontiguousarray(
        wt.reshape(groups, 128, m).transpose(1, 0, 2).reshape(128, groups * m))


def _make_in_maps(x, q_w, kv_w, proj_w):
    x = np.asarray(x, dtype=np.float32)
    q_w = np.asarray(q_w, dtype=np.float32)
    kv_w = np.asarray(kv_w, dtype=np.float32)
    proj_w = np.asarray(proj_w, dtype=np.float32)
    f16 = np.float16
    in_maps = []
    for core in range(NCORES):
        b, g = core // 2, core % 2
        hs = slice(g * 256, (g + 1) * 256)
        in_maps.append({
            "xT": np.ascontiguousarray(x[b].T.astype(f16)),
            "wqT": _pack((q_w[hs, :] * np.float32(SCALE)).T.astype(f16), CT),
            "wkT": _pack(kv_w[hs, :].T.astype(f16), CT),
            "wvT": _pack(
                kv_w[C + g * 256:C + (g + 1) * 256, :].T.astype(f16), CT),
            "pwT": _pack(proj_w[:, hs].T.astype(f16), 2),
        })
    return in_maps


def kernel(x, q_w, kv_w, proj_w, proj_b, H=None, W=None, _trace=False):
    from concourse.bass_utils import run_bass_kernel_spmd

    nc = _get_nc()
    in_maps = _make_in_maps(x, q_w, kv_w, proj_w)
    res = run_bass_kernel_spmd(nc, in_maps, core_ids=list(range(NCORES)),
                               trace=_trace)
    proj_b = np.asarray(proj_b, dtype=np.float32)
    out = np.empty((B, N, C), dtype=np.float32)
    for b in range(B):
        out[b] = res.results[2 * b]["y"] + res.results[2 * b + 1]["y"] + proj_b
    if _trace:
        return out, res
    return out


# revision 35
# speedup vs baseline: 1.0356x; 1.0113x over previous
"""Trainium2 Bass kernel for multi-head attention (B=4, N=2048, C=512, 8 heads).

Sharding: 8 cores = (batch b = core//2) x (head-group g = core%2, 4 heads each).
Per core, a transposed-scores attention pipeline:
  - host supplies x[b] transposed (xT [C, N]) and per-group transposed weights,
    all pre-cast to fp16 (matmul streams at 1 cycle/row; ~4x the mantissa of
    bf16; every tensor here fits fp16 range comfortably)
  - qT/kT stored zero-padded per head ([:, hh, :] has head hh's 64 dims on
    its own partition range, rest zero) so score matmuls contract over the
    full K=128 partition range: same N cycles as K=64, but the PE activity
    monitor sees a fully-active array and keeps the 2.4 GHz clock
  - v as [N, (1+64) per head] tiles; the leading ones column makes attn@v
    emit the softmax denominator into PSUM partition 0
  - 8 single-head sections of 16 ktok blocks. The section phase is ACT-bound
    (exp is ScalarE-only, ~1.06us per [128,1024] block, 128 blocks = ~136us
    floor) and total PE work ~= total ACT work, so the schedule's only job
    is keeping BOTH streams dense:
      * software-pipelined blocks: each iteration emits exp(i) then
        scores(i+1) then attnv(i-1) then fillers, so the next block's
        scores (the exp stream's input) is never stuck behind attnv or
        filler work, and PSUM bank waits self-correct
      * DMA loads ordered by first use (wk, xT half, wv, wq, ...), one
        trigger per token range
      * critical prefix is only kT chunk 0 + v tile 0 + qT chunks 0,1;
        everything else trickles in as deadline-scheduled fillers at
        half-chunk (256-token) granularity
      * sections run pair-grouped so pair-1 qk chunks have 3 sections of
        filler slack, and the first half of the output projection runs as
        fillers inside the last two sections
  - normalization off the PE in 512-wide chunks, recips emitted before
    broadcasts so the two chunks pipeline across DVE/GpSimd; the last
    section's chunk-0 chain gates the tail y blocks
  - tail y blocks split their PSUM eviction across ACT+DVE so the 2-deep
    PSUM ring turns over at matmul rate
  - output projection on-device; host sums the two half-head partials
"""

import sys

sys.path.insert(0, "/opt/trn_rl_repo")

import numpy as np

B, N, C = 4, 2048, 512
H, D = 8, 64
SCALE = float(D) ** -0.5  # 0.125, exact in fp32
P = 128
CT = C // P  # 4 contraction tiles over channels
NT = N // P  # 16 token blocks
NCORES = 8
FD = 1024  # softmax block free dim (q chunk)
QH = N // FD  # 2 q halves

_cache = {}


def _build():
    import concourse.bacc as bacc
    import concourse.tile as tile
    from concourse import mybir

    f32 = mybir.dt.float32
    f16 = mybir.dt.float16
    u16 = mybir.dt.uint16
    EXP = mybir.ActivationFunctionType.Exp

    nc = bacc.Bacc("TRN2", target_bir_lowering=False, debug=False,
                   num_devices=NCORES)

    xT_d = nc.dram_tensor("xT", [C, N], f16, kind="ExternalInput")
    wqT_d = nc.dram_tensor("wqT", [P, CT * 256], f16, kind="ExternalInput")
    wkT_d = nc.dram_tensor("wkT", [P, CT * 256], f16, kind="ExternalInput")
    wvT_d = nc.dram_tensor("wvT", [P, CT * 256], f16, kind="ExternalInput")
    pwT_d = nc.dram_tensor("pwT", [P, 2 * C], f16, kind="ExternalInput")
    y_d = nc.dram_tensor("y", [N, C], f32, kind="ExternalOutput")

    with tile.TileContext(nc) as tc:
        with (
            tc.tile_pool(name="io", bufs=1) as io,
            tc.tile_pool(name="qk", bufs=1) as qk,
            tc.tile_pool(name="expp", bufs=6) as expp,
            tc.tile_pool(name="workp", bufs=3) as workp,
            tc.tile_pool(name="yp", bufs=4) as yp,
            tc.tile_pool(name="ps_s", bufs=2, space="PSUM") as ps_s,
            tc.tile_pool(name="ps_o", bufs=1, space="PSUM") as ps_o,
        ):
            # ---- input loads, ordered by first use ----
            xT_sb = io.tile([P, CT, N], f16, tag="xT", name="xT_sb")
            xT_ap = xT_d[:].rearrange("(t p) n -> p t n", p=P)
            wk_sb = io.tile([P, CT, 256], f16, tag="wk", name="wk_sb")
            nc.sync.dma_start(
                wk_sb[:], wkT_d[:].rearrange("p (t m) -> p t m", t=CT))
            nc.sync.dma_start(xT_sb[:, :, 0:256], xT_ap[:, :, 0:256])
            wq_sb = io.tile([P, CT, 256], f16, tag="wq", name="wq_sb")
            nc.sync.dma_start(
                wq_sb[:], wqT_d[:].rearrange("p (t m) -> p t m", t=CT))
            nc.sync.dma_start(xT_sb[:, :, 256:512], xT_ap[:, :, 256:512])
            # bulk xT in t-split pairs so two DMA queues stream concurrently;
            # 512:1024 quarters land first (they gate the prefix q chunks)
            nc.sync.dma_start(xT_sb[:, 0:2, 512:1024], xT_ap[:, 0:2, 512:1024])
            nc.sync.dma_start(xT_sb[:, 2:4, 512:1024], xT_ap[:, 2:4, 512:1024])
            nc.sync.dma_start(xT_sb[:, 0:2, 1024:1536], xT_ap[:, 0:2, 1024:1536])
            nc.sync.dma_start(xT_sb[:, 2:4, 1024:1536], xT_ap[:, 2:4, 1024:1536])
            wv_sb = io.tile([P, CT, 256], f16, tag="wv", name="wv_sb")
            nc.sync.dma_start(
                wv_sb[:], wvT_d[:].rearrange("p (t m) -> p t m", t=CT))
            nc.sync.dma_start(xT_sb[:, 0:2, 1536:2048], xT_ap[:, 0:2, 1536:2048])
            nc.sync.dma_start(xT_sb[:, 2:4, 1536:2048], xT_ap[:, 2:4, 1536:2048])
            pw_sb = io.tile([P, 2, C], f16, tag="pw", name="pw_sb")
            nc.sync.dma_start(
                pw_sb[:], pwT_d[:].rearrange("p (t m) -> p t m", t=2))

            # ---- SBUF persistents ----
            qT = []
            kT = []
            vv = []
            outT = []
            for p in range(2):
                qT.append(qk.tile([P, 2, N], f16, tag=f"qT{p}", name=f"qT{p}"))
                kT.append(qk.tile([P, 2, N], f16, tag=f"kT{p}", name=f"kT{p}"))
                vv.append(qk.tile([P, NT, 130], f16, tag=f"v{p}", name=f"v{p}"))
                outT.append(qk.tile([P, N], f16, tag=f"outT{p}", name=f"outT{p}"))

            # trigger the ACT exp table load during the DMA ramp
            scratch1 = io.tile([1, 2], f32, tag="scratch1", name="scratch1")
            nc.vector.memset(scratch1[:], 0.0)
            nc.scalar.activation(scratch1[0:1, 0:1], scratch1[0:1, 1:2], EXP)
            # all pads on GpSimd (its queue has nothing else early, so they
            # run right at kernel start; DVE stays free for the prefix
            # evictions), slot-0 halves first
            nc.gpsimd.memset(kT[0][64:128, 0, :], 0.0)
            nc.gpsimd.memset(qT[0][64:128, 0, :], 0.0)
            nc.gpsimd.memset(kT[0][0:64, 1, :], 0.0)
            nc.gpsimd.memset(qT[0][0:64, 1, :], 0.0)
            for p in range(2):
                # ones columns (fp16 1.0) at the head of each v block (DVE:
                # strided 16-element memsets are fast there, slow on GpSimd)
                nc.vector.memset(vv[p][:, :, 0:1].bitcast(u16), 0x3C00)
                nc.vector.memset(vv[p][:, :, 65:66].bitcast(u16), 0x3C00)

            # PSUM layout: tag "s" = the scores<->exp ring (2x2 banks), tag
            # "f" = everything else (prefix chunks, v tiles, y blocks; 2x1
            # bank), tag "o" = the attn@v accumulator (1x2 banks). Separate
            # rings keep filler eviction latency out of the exp pipeline.
            def emit_qk_chunk(p, w_sb, dst, cs, act_evict=False, ring="f"):
                # cs: token slice (512-wide prefix chunks, 256-wide fillers)
                cw = cs.stop - cs.start
                pc = slice(128 * p, 128 * (p + 1))
                shape = [P, 512] if ring == "f" else [P, FD]
                ps = ps_s.tile(shape, f32, tag=ring,
                               name=f"qkps_{p}_{cs.start}_{w_sb.tensor.name}")
                for t in range(CT):
                    nc.tensor.matmul(
                        ps[:, :cw],
                        lhsT=w_sb[:, t, pc],
                        rhs=xT_sb[:, t, cs],
                        start=(t == 0), stop=(t == CT - 1))
                nc.vector.tensor_copy(dst[0:64, 0, cs], ps[0:64, :cw])
                if act_evict:
                    nc.scalar.copy(dst[64:128, 1, cs], ps[64:128, :cw])
                else:
                    nc.vector.tensor_copy(dst[64:128, 1, cs], ps[64:128, :cw])

            def emit_v_tile(tt):
                psv = ps_s.tile([P, 512], f32, tag="f", name=f"vps_{tt}")
                for t in range(CT):
                    nc.tensor.matmul(
                        psv[:, :256],
                        lhsT=xT_sb[:, t, 128 * tt:128 * (tt + 1)],
                        rhs=wv_sb[:, t, 0:256],
                        start=(t == 0), stop=(t == CT - 1))
                for p in range(2):
                    pv = psv[:, 128 * p:128 * (p + 1)].rearrange(
                        "p (two d) -> p two d", two=2)
                    dv = vv[p][:, tt, 0:130].rearrange(
                        "p (two d65) -> p two d65", two=2)[:, :, 1:65]
                    nc.vector.tensor_copy(dv, pv)

            def emit_v_pair(tt):
                # two v tiles in one PSUM allocation, evicted with one DVE
                # copy per pair (halves eviction count and ring slots)
                psv = ps_s.tile([P, 512], f32, tag="f", name=f"vps_{tt}")
                for half in range(2):
                    for t in range(CT):
                        nc.tensor.matmul(
                            psv[:, 256 * half:256 * (half + 1)],
                            lhsT=xT_sb[:, t, 128 * (tt + half):
                                       128 * (tt + half + 1)],
                            rhs=wv_sb[:, t, 0:256],
                            start=(t == 0), stop=(t == CT - 1))
                pv4 = psv.rearrange("pp (tile pr two d) -> pp tile pr two d",
                                    tile=2, pr=2, two=2)
                for p in range(2):
                    dv = vv[p][:, tt:tt + 2, 0:130].rearrange(
                        "pp tile (two d65) -> pp tile two d65",
                        two=2)[:, :, :, 1:65]
                    nc.vector.tensor_copy(dv, pv4[:, :, p, :, :])

            def emit_y_block(tt, mode):
                yps = ps_s.tile([P, 512], f32, tag="f", name=f"yps_{tt}")
                for p in range(2):
                    nc.tensor.matmul(
                        yps[:, :512], lhsT=outT[p][:, 128 * tt:128 * (tt + 1)],
                        rhs=pw_sb[:, p, :], start=(p == 0), stop=(p == 1))
                ys = yp.tile([P, C], f32, tag="y", name=f"ys_{tt}")
                if mode == "dual":
                    # tail: split across ACT+DVE so the PSUM ring turns over
                    # at matmul rate
                    nc.scalar.copy(ys[:, 0:256], yps[:, 0:256])
                    nc.vector.tensor_copy(ys[:, 256:512], yps[:, 256:512])
                elif mode == "act":
                    nc.scalar.copy(ys[:], yps[:, :512])
                else:
                    nc.vector.tensor_copy(ys[:], yps[:, :512])
                nc.sync.dma_start(y_d[128 * tt:128 * (tt + 1), :], ys[:])

            def norm_head(p, hh, qh, o, last=False):
                # evict o to SBUF immediately (frees the single-buffered o
                # bank for the next section's attn@v), then normalize from
                # the copy: DVE recips -> GpSimd broadcasts -> DVE muls.
                # The last section skips the staging copy (its o bank has no
                # next user) and runs 256-wide chunks so the first tail y
                # blocks' outT slice lands as early as possible.
                if last:
                    oc = o
                    nch, cw = 4, 256
                else:
                    oc = workp.tile([65, FD], f32, tag="oc",
                                    name=f"oc_{p}_{hh}_{qh}")
                    nc.vector.tensor_copy(oc[:], o[:])
                    nch, cw = 2, 512
                rs = []
                for c in range(nch):
                    r = workp.tile([1, 512], f32, tag="r",
                                   name=f"r_{p}_{hh}_{qh}_{c}", bufs=4)
                    nc.vector.reciprocal_approx_fast(
                        r[0:1, :cw], oc[0:1, cw * c:cw * (c + 1)])
                    rs.append(r)
                rbs = []
                for c in range(nch):
                    rb = workp.tile([65, 512], f32, tag="rb",
                                    name=f"rb_{p}_{hh}_{qh}_{c}", bufs=4)
                    nc.gpsimd.partition_broadcast(rb[:, :cw], rs[c][0:1, :cw])
                    rbs.append(rb)
                for c in range(nch):
                    qs = slice(FD * qh + cw * c, FD * qh + cw * (c + 1))
                    st = workp.tile([65, 512], f16, tag="st",
                                    name=f"st_{p}_{hh}_{qh}_{c}", bufs=4)
                    nc.vector.tensor_mul(
                        st[:, :cw], oc[:, cw * c:cw * (c + 1)], rbs[c][:, :cw])
                    nc.sync.dma_start(outT[p][64 * hh:64 * (hh + 1), qs],
                                      st[1:65, :cw])

            # deadline-scheduled fillers: (section_idx, block_idx) -> [fn]
            fill = {}

            def add_fill(si, blk, fn):
                fill.setdefault((si, blk), []).append(fn)

            def emit_section(si, p, hh, qh, last=False):
                vs = slice(65 * hh, 65 * (hh + 1))
                o = ps_o.tile([65, FD], f32, tag="o", name=f"o_{p}_{hh}_{qh}")

                def emit_scores(i):
                    ks = slice(128 * i, 128 * (i + 1))
                    s = ps_s.tile([P, FD], f32, tag="s",
                                  name=f"s_{p}_{hh}_{qh}_{i}")
                    for j in range(2):
                        js = slice(512 * j, 512 * (j + 1))
                        qj = slice(FD * qh + 512 * j, FD * qh + 512 * (j + 1))
                        nc.tensor.matmul(
                            s[:, js], lhsT=kT[p][:, hh, ks],
                            rhs=qT[p][:, hh, qj], start=True, stop=True)
                    return s

                def emit_attnv(i, e):
                    for j in range(2):
                        js = slice(512 * j, 512 * (j + 1))
                        nc.tensor.matmul(
                            o[:, js], lhsT=vv[p][:, i, vs], rhs=e[:, js],
                            start=(i == 0), stop=(i == NT - 1))

                s_cur = emit_scores(0)
                e_prev = None
                for i in range(NT):
                    e_cur = expp.tile([P, FD], f16, tag="exp",
                                      name=f"e_{p}_{hh}_{qh}_{i}")
                    nc.scalar.activation(e_cur[:], s_cur[:], EXP)
                    if i + 1 < NT:
                        s_cur = emit_scores(i + 1)
                    if e_prev is not None:
                        emit_attnv(i - 1, e_prev)
                    e_prev = e_cur
                    for fn in fill.pop((si, i), []):
                        fn()
                emit_attnv(NT - 1, e_prev)

                norm_head(p, hh, qh, o, last=last)

            # critical prefix: only what section (0,0,0) block 0 touches,
            # quarter-chunk granularity ordered by DMA arrival. scores
            # block 0 only needs kT tokens 0:128; the rest of kT[0] fills
            # in deadline order inside section 0.
            # prefix chunks alternate PSUM rings (f/s — scores haven't
            # started, so the s ring is free) so each chunk's matmuls never
            # wait on the immediately preceding chunk's evictions
            emit_qk_chunk(0, wk_sb, kT[0], slice(0, 128))
            emit_qk_chunk(0, wq_sb, qT[0], slice(0, 256), ring="s")
            emit_qk_chunk(0, wk_sb, kT[0], slice(128, 384))
            emit_qk_chunk(0, wq_sb, qT[0], slice(256, 512), ring="s")
            emit_qk_chunk(0, wq_sb, qT[0], slice(512, 768))
            emit_qk_chunk(0, wq_sb, qT[0], slice(768, 1024), ring="s")
            emit_v_tile(0)

            def hc(p, w_sb, dst, h):  # 256-token half-chunk filler
                return lambda: emit_qk_chunk(
                    p, w_sb, dst, slice(256 * h, 256 * (h + 1)))

            # section 0: v tiles 1-15 (tile tt consumed by attnv at block
            # tt+1); kT[0] pieces (tokens 256h+128, needed by scores block
            # 2h+1 which is emitted at iteration 2h) placed at iteration
            # 2h-1
            for tt in range(1, 14, 2):
                add_fill(0, tt, lambda tt=tt: emit_v_pair(tt))
            add_fill(0, 14, lambda: emit_v_tile(15))
            for h in range(1, 7):
                add_fill(0, 2 * h - 2, lambda h=h: emit_qk_chunk(
                    0, wk_sb, kT[0], slice(256 * h + 128, 256 * h + 384)))
            add_fill(0, 12, lambda: emit_qk_chunk(
                0, wk_sb, kT[0], slice(1920, 2048)))
            # section 1: qT[0] tokens 1024:2048 (needed by section 2 =
            # (0,0,1)), pair-1 pads (gpsimd), first pair-1 k half-chunks
            add_fill(1, 1, lambda: nc.gpsimd.memset(kT[1][64:128, 0, :], 0.0))
            add_fill(1, 2, lambda: nc.gpsimd.memset(kT[1][0:64, 1, :], 0.0))
            add_fill(1, 3, lambda: nc.gpsimd.memset(qT[1][64:128, 0, :], 0.0))
            add_fill(1, 4, lambda: nc.gpsimd.memset(qT[1][0:64, 1, :], 0.0))
            for j, h in enumerate(range(4, 8)):
                add_fill(1, 2 + 3 * j, hc(0, wq_sb, qT[0], h))
            add_fill(1, 13, hc(1, wk_sb, kT[1], 0))
            add_fill(1, 15, hc(1, wk_sb, kT[1], 1))
            # sections 2-3: rest of pair-1 (deadline: section 4 start)
            for j, h in enumerate(range(2, 8)):
                add_fill(2, 1 + 2 * j, hc(1, wk_sb, kT[1], h))
            add_fill(2, 13, hc(1, wq_sb, qT[1], 0))
            add_fill(2, 15, hc(1, wq_sb, qT[1], 1))
            for j, h in enumerate(range(2, 8)):
                add_fill(3, 1 + 2 * j, hc(1, wq_sb, qT[1], h))
            # sections 6-7: y blocks 0-7 (qh=0 outT complete after section
            # 5's norm chain ~5us into section 6 — placed mid-section so the
            # in-order PE never parks on a not-yet-normalized outT; DVE
            # evictions keep ACT pure-exp)
            for i in range(4):
                add_fill(6, 6 + 3 * i, lambda tt=i: emit_y_block(tt, "dve"))
            for i in range(4, 8):
                add_fill(7, 2 + 3 * (i - 4),
                         lambda tt=i: emit_y_block(tt, "dve"))

            # pair-grouped section order: pair-1 qk chunks get 3 sections of
            # filler slack; last two sections host the first 8 y blocks
            sections = [(0, 0, 0), (0, 1, 0), (0, 0, 1), (0, 1, 1),
                        (1, 0, 0), (1, 1, 0), (1, 0, 1), (1, 1, 1)]
            for si, (p, hh, qh) in enumerate(sections):
                emit_section(si, p, hh, qh, last=(si == len(sections) - 1))
            for key in sorted(fill.keys()):
                for fn in fill.pop(key):
                    fn()

            # ---- tail: y blocks 8-15 ----
            for tt in range(8, NT):
                emit_y_block(tt, "dual")

    nc.finalize()
    return nc


def _get_nc():
    if "nc" not in _cache:
        _cache["nc"] = _build()
    return _cache["nc"]


def _pack(wt, groups):
    # [G*128, M] row-major -> [128, G*M]: partition p holds the concat over
    # groups of row (g*128 + p), so the DMA reads one contiguous run per p
    g128, m = wt.shape
    assert g128 == groups * 128
    return np.ascontiguousarray(
        wt.reshape(groups, 128, m).transpose(1, 0, 2).reshape(128, groups * m))


def _make_in_maps(x, q_w, kv_w, proj_w):
    x = np.asarray(x, dtype=np.float32)
    q_w = np.asarray(q_w, dtype=np.float32)
    kv_w = np.asarray(kv_w, dtype=np.float32)
    proj_w = np.asarray(proj_w, dtype=np.float32)
    f16 = np.float16
    in_maps = []
    for core in range(NCORES):
        b, g = core // 2, core % 2
        hs = slice(g * 256, (g + 1) * 256)
        in_maps.append({
            "xT": np.ascontiguousarray(x[b].T.astype(f16)),
            "wqT": _pack((q_w[hs, :] * np.float32(SCALE)).T.astype(f16), CT),
            "wkT": _pack(kv_w[hs, :].T.astype(f16), CT),
            "wvT": _pack(
                kv_w[C + g * 256:C + (g + 1) * 256, :].T.astype(f16), CT),
            "pwT": _pack(proj_w[:, hs].T.astype(f16), 2),
        })
    return in_maps


def kernel(x, q_w, kv_w, proj_w, proj_b, H=None, W=None, _trace=False):
    from concourse.bass_utils import run_bass_kernel_spmd

    nc = _get_nc()
    in_maps = _make_in_maps(x, q_w, kv_w, proj_w)
    res = run_bass_kernel_spmd(nc, in_maps, core_ids=list(range(NCORES)),
                               trace=_trace)
    proj_b = np.asarray(proj_b, dtype=np.float32)
    out = np.empty((B, N, C), dtype=np.float32)
    for b in range(B):
        out[b] = res.results[2 * b]["y"] + res.results[2 * b + 1]["y"] + proj_b
    if _trace:
        return out, res
    return out
